# revision 6
# baseline (speedup 1.0000x reference)
"""MixHop layer (3 hops) on 8 Trainium2 NeuronCores.

out = concat_j [ adj_t^j @ (x @ W_j.T + b_j) ]   for j = 0,1,2

Strategy (destination sharding, one SPMD program on 8 cores):
  - Each core owns N/8 destination rows and the edges pointing into them
    (edges grouped on the host into degree-balanced blocks of 128 dests).
  - Phase A: y0 = x@W0.T+b0 for the own shard (fp32).
  - Phase B: every core (redundantly) projects x -> [y1|y2] table [N,256]
    fp16 with PE matmuls (x passed host-transposed and host-cast to fp16).
  - Phase C (SpMM1): dma_gather 512B fp16 table rows per in-edge (block-major
    global chunk stream, <=1024 ids per gather), build the one-hot*weight
    segment matrix S on device (one tensor_scalar is_equal+mult against an
    iota tile), segment-sum via PE matmuls accumulated in PSUM.  Cols 0:128
    -> out1 (fp32), cols 128:256 -> z2 shard (fp16); both written with
    batched dma_scatter_add into pre-zeroed buffers.
  - Phase D: AllGather z2 shards -> full z2 table [N,128] fp16.
  - Phase E (SpMM2): same edge structure gathers z2 -> out2 (fp32).
All per-core variation (indices, segment data, scatter rows) is carried as
input data so a single program serves all cores.  fp16 is used only for
gather-table payloads and the segment matrix (PSUM accumulation stays fp32);
measured end-to-end error vs the fp32 reference is ~5e-4 max-rel.
"""

import sys

sys.path.insert(0, "/opt/trn_rl_repo")

import heapq
import os

import numpy as np

import concourse.bass as bass
import concourse.tile as tile
from concourse import bacc, mybir
from concourse import bass_utils

P = 128


class Cfg:
    def __init__(self, n_nodes, n_feat, n_cores, k0max, k1max):
        assert n_nodes % n_cores == 0
        self.N = n_nodes
        self.F = n_feat
        self.NC = n_cores
        self.NS = n_nodes // n_cores          # dests per core
        self.NBLK = -(-self.NS // P)          # blocks per core
        self.K0 = k0max                       # window-0 chunks per block
        self.K1 = k1max                       # window-1 chunks per block
        self.K = k0max + k1max
        self.GMAX = 8                         # chunks per dma_gather (<=1024 ids)
        self.SGRP = 8                         # blocks per dma_scatter_add
        self.NSG = -(-self.NBLK // self.SGRP)
        self.NG0 = -(-(self.NBLK * k0max) // self.GMAX)   # win0 gathers/pass
        self.NG1 = -(-(self.NBLK * k1max) // self.GMAX)
        self.WIN = 32768 if n_nodes > 32768 else max(P, n_nodes // 2)
        self.NPAD = -(-n_nodes // 512) * 512  # table rows (512-aligned)


def _balanced_blocks(local_dest, ns, nblk):
    """Assign dests 0..ns-1 to nblk blocks of <=P slots, balancing edge
    counts.  Returns (block_of[ns], pos_of[ns], ids[P, nblk])."""
    deg = np.bincount(local_dest, minlength=ns)
    order = np.argsort(-deg, kind="stable")
    heap = [(0, 0, b) for b in range(nblk)]
    heapq.heapify(heap)
    block_of = np.empty(ns, np.int32)
    pos_of = np.empty(ns, np.int32)
    for d in order:
        while True:
            load, cnt, b = heapq.heappop(heap)
            if cnt < P:
                break
        block_of[d] = b
        pos_of[d] = cnt
        heapq.heappush(heap, (load + int(deg[d]), cnt + 1, b))
    # slot p of block b -> local output row (trash rows ns+p for empty slots)
    ids = np.empty((P, nblk), np.int32)
    for p in range(P):
        ids[p, :] = ns + p
    ids[pos_of, block_of] = np.arange(ns, dtype=np.int32)
    return block_of, pos_of, ids


def _precompute_core(r_loc, c_glob, w, cfg):
    ns, nblk = cfg.NS, cfg.NBLK
    block_of, pos_of, ids = _balanced_blocks(r_loc, ns, nblk)
    b_e = block_of[r_loc]
    dl_e = pos_of[r_loc]
    win_e = (c_glob >= cfg.WIN).astype(np.int64)
    order = np.lexsort((np.arange(len(r_loc)), win_e, b_e))
    b_s, win_s, dl_s, c_s, w_s = (
        b_e[order], win_e[order], dl_e[order], c_glob[order], w[order])
    key = b_s * 2 + win_s
    cnt = np.bincount(key, minlength=nblk * 2).reshape(nblk, 2)
    k0need = max(1, int(np.ceil(cnt[:, 0].max() / P))) if len(r_loc) else 1
    k1need = max(1, int(np.ceil(cnt[:, 1].max() / P))) if len(r_loc) else 1
    return dict(b=b_s, win=win_s, dl=dl_s, c=c_s, w=w_s, cnt=cnt, ids=ids,
                k0=k0need, k1=k1need)


def _encode_core(pc, cfg):
    """Device input arrays for one core, given global K0/K1."""
    nblk, K0, K1, K = cfg.NBLK, cfg.K0, cfg.K1, cfg.K
    cnt = pc["cnt"]
    idx0 = np.zeros((nblk, K0 * P), np.int16)     # padded edge ids (win0)
    idx1 = np.zeros((nblk, K1 * P), np.int16)
    meta = np.zeros((P, nblk, K, 2), np.float32)  # (local dest, weight)
    starts = np.zeros(nblk * 2, np.int64)
    starts[1:] = np.cumsum(cnt.reshape(-1))[:-1]
    key = pc["b"] * 2 + pc["win"]
    iw = np.arange(len(key)) - starts[key]        # index within (b, win)
    b, win, dl, c, w = pc["b"], pc["win"], pc["dl"], pc["c"], pc["w"]
    m0 = win == 0
    idx0[b[m0], iw[m0]] = c[m0].astype(np.int16)
    m1 = ~m0
    idx1[b[m1], iw[m1]] = (c[m1] - cfg.WIN).astype(np.int16)
    kk = np.where(m0, iw // P, K0 + iw // P)
    meta[iw % P, b, kk, 0] = dl
    meta[iw % P, b, kk, 1] = w

    # global chunk-stream gather encodings [128, n_gath*GMAX*8]; dma_gather
    # reads logical id i from [i%16, i//16] of its idx window, replicated to
    # all 8 GPSIMD core groups (partitions 0..127).
    GM = cfg.GMAX

    def enc(idx, Kw, n_gath):
        stream = idx.reshape(nblk * Kw * P)
        out = np.zeros((P, n_gath, GM * 8), np.int16)
        for g in range(n_gath):
            cg = min(GM, nblk * Kw - GM * g)
            flat = stream[g * GM * P: g * GM * P + cg * P]
            e = flat.reshape(-1, 16).T
            out[:, g, :cg * 8] = np.tile(e, (8, 1))
        return out.reshape(P, n_gath * GM * 8)

    # batched scatter ids: group g covers SGRP blocks; logical i = c*128+p
    ids = pc["ids"]
    sid = np.zeros((P, cfg.NSG, cfg.SGRP * 8), np.int16)
    for g in range(cfg.NSG):
        nb = min(cfg.SGRP, nblk - g * cfg.SGRP)
        flat = ids[:, g * cfg.SGRP: g * cfg.SGRP + nb].T.reshape(-1)
        e = flat.reshape(-1, 16).T.astype(np.int16)
        sid[:, g, :nb * 8] = np.tile(e, (8, 1))
    return dict(
        idx0=enc(idx0, K0, cfg.NG0), idx1=enc(idx1, K1, cfg.NG1),
        meta=np.ascontiguousarray(meta.reshape(P, nblk * K * 2)),
        sid=np.ascontiguousarray(sid.reshape(P, cfg.NSG * cfg.SGRP * 8)),
    )


def _build_program(cfg, phases="ABCDE"):
    N, F, NC = cfg.N, cfg.F, cfg.NC
    NS, NBLK, K0, K1, K = cfg.NS, cfg.NBLK, cfg.K0, cfg.K1, cfg.K
    NW0 = min(N, cfg.WIN)
    NSP = NS + P                             # out buf rows incl trash
    NTILE_Y0 = NBLK
    NTILE_TAB = -(-N // P)
    f32 = mybir.dt.float32
    f16 = mybir.dt.float16
    GM, NG0, NG1 = cfg.GMAX, cfg.NG0, cfg.NG1
    SG, NSG = cfg.SGRP, cfg.NSG

    nc = bacc.Bacc("TRN2", target_bir_lowering=False, debug=False,
                   enable_asserts=False, num_devices=NC, num_swdge_queues=4)

    # ---- inputs ----------------------------------------------------------
    NP_ = cfg.NPAD
    xT16 = nc.dram_tensor("xT16", [F, NP_], f16, kind="ExternalInput").ap()
    xsT = nc.dram_tensor("xsT", [F, NBLK * P], f16, kind="ExternalInput").ap()
    WT = nc.dram_tensor("WT", [3 * F, F], f16, kind="ExternalInput").ap()
    BB = nc.dram_tensor("BB", [3 * P, F], f32, kind="ExternalInput").ap()
    B16 = nc.dram_tensor("B16", [3, F], f16, kind="ExternalInput").ap()
    iota_in = nc.dram_tensor("iota", [P, P], f16, kind="ExternalInput").ap()
    idx0_in = nc.dram_tensor("idx0", [P, NG0 * GM * 8], mybir.dt.int16,
                             kind="ExternalInput").ap()
    idx1_in = nc.dram_tensor("idx1", [P, NG1 * GM * 8], mybir.dt.int16,
                             kind="ExternalInput").ap()
    meta_in = nc.dram_tensor("meta", [P, NBLK * K * 2], f32,
                             kind="ExternalInput").ap()
    sid_in = nc.dram_tensor("sid", [P, NSG * SG * 8], mybir.dt.int16,
                            kind="ExternalInput").ap()

    # ---- outputs / scratch ----------------------------------------------
    y0_buf = nc.dram_tensor("y0", [NBLK * P, F], f32, kind="ExternalOutput").ap()
    out1_buf = nc.dram_tensor("out1", [NSP, F], f32, kind="ExternalOutput").ap()
    out2_buf = nc.dram_tensor("out2", [NSP, F], f32, kind="ExternalOutput").ap()
    table = nc.dram_tensor("table", [NP_, 2 * F], f16, kind="Internal").ap()
    z2s = nc.dram_tensor("z2s", [NSP, F], f16, kind="Internal").ap()
    z2t = nc.dram_tensor("z2t", [N, F], f16, kind="Internal",
                         addr_space="Shared").ap()

    with tile.TileContext(nc) as tc:
        with tc.tile_pool(name="const", bufs=1) as cpool:
            iota_t = cpool.tile([P, P], f16)
            nc.sync.dma_start(iota_t[:], iota_in[:])
            meta_t = cpool.tile([P, NBLK * K * 2], f32)
            nc.sync.dma_start(meta_t[:], meta_in[:])
            ix0_t = cpool.tile([P, NG0 * GM * 8], mybir.dt.int16)
            nc.sync.dma_start(ix0_t[:], idx0_in[:])
            ix1_t = cpool.tile([P, NG1 * GM * 8], mybir.dt.int16)
            nc.sync.dma_start(ix1_t[:], idx1_in[:])
            sid_t = cpool.tile([P, NSG * SG * 8], mybir.dt.int16)
            nc.sync.dma_start(sid_t[:], sid_in[:])
            wt_t = []
            bb_t = []
            for j in range(3):
                wtj = cpool.tile([F, F], f16, tag=f"wt{j}", name=f"wt{j}")
                bbj = cpool.tile([P, F], f32, tag=f"bb{j}", name=f"bb{j}")
                nc.sync.dma_start(wtj[:], WT[j * F:(j + 1) * F, :])
                nc.sync.dma_start(bbj[:], BB[j * P:(j + 1) * P, :])
                wt_t.append(wtj)
                bb_t.append(bbj)
            b16_t = []
            for j in range(3):
                b16j = cpool.tile([1, F], f16, tag=f"b16{j}", name=f"b16{j}")
                nc.sync.dma_start(b16j[:], B16[j:j + 1, :])
                b16_t.append(b16j)
            ones_t = cpool.tile([1, P], f16)
            nc.vector.memset(ones_t[:], 1.0)

            # ---- zero z2s (scatter-add base) -----------------------------
            if "C" in phases:
                with tc.tile_pool(name="zz", bufs=1) as zpool:
                    zt = zpool.tile([P, 2048], f16)
                    nc.vector.memset(zt[:], 0.0)
                    nrow = 0
                    while nrow + 2048 <= NSP:
                        nc.sync.dma_start(
                            z2s[nrow:nrow + 2048, :].rearrange(
                                "(a b) f -> a (b f)", a=P), zt[:])
                        nrow += 2048
                    while nrow + P <= NSP:
                        nc.sync.dma_start(
                            z2s[nrow:nrow + P, :].rearrange(
                                "(a b) f -> a (b f)", a=P), zt[:, :F])
                        nrow += P
                    assert nrow >= NS, (nrow, NS)

            # ---- Phase A: y0 = xs@W0.T + b0 (own shard, fp32 out) --------
            if "A" in phases:
             with tc.tile_pool(name="projA", bufs=3) as apool, \
                  tc.tile_pool(name="psumA", bufs=3, space="PSUM") as apsum:
                for t in range(NTILE_Y0):
                    r0 = t * P
                    r1 = min(NS, r0 + P)
                    w_ = r1 - r0
                    if w_ <= 0:
                        break
                    xt = apool.tile([F, P], f16, tag="xt")
                    nc.sync.dma_start(xt[:, :w_], xsT[:, r0:r1])
                    ps0 = apsum.tile([P, F], f32, space="PSUM")
                    nc.tensor.matmul(ps0[:w_, :], lhsT=xt[:, :w_],
                                     rhs=wt_t[0][:], start=True, stop=True)
                    st0 = apool.tile([P, F], f32, tag="st0")
                    nc.vector.tensor_tensor(out=st0[:w_, :], in0=ps0[:w_, :],
                                            in1=bb_t[0][:w_, :],
                                            op=mybir.AluOpType.add)
                    nc.sync.dma_start(y0_buf[r0:r1, :], st0[:w_, :])

            # ---- Phase B: full fp16 table (replicated per core) ----------
            # 512-node groups: one wide load, 4x2 matmuls (+rank-1 bias
            # matmul), one wide store.  psum->staging copies alternate
            # DVE/ACT to spread engine load.
            if "B" in phases:
             NGRP_B = NP_ // 512
             with tc.tile_pool(name="projB", bufs=3) as bpool, \
                  tc.tile_pool(name="psumB", bufs=4, space="PSUM") as bpsum:
                for t in range(NGRP_B):
                    r0 = t * 512
                    r1 = r0 + 512
                    gw = 512
                    xt = bpool.tile([F, 512], f16, tag="xtb")
                    nc.sync.dma_start(xt[:, :gw], xT16[:, r0:r1])
                    st = bpool.tile([P, 4, 2 * F], f16, tag="stb")
                    nsub = -(-gw // P)
                    for s in range(nsub):
                        c0 = s * P
                        w_ = min(P, gw - c0)
                        ps = bpsum.tile([P, 2 * F], f32, space="PSUM")
                        for j in (1, 2):
                            nc.tensor.matmul(
                                ps[:w_, (j - 1) * F:j * F],
                                lhsT=xt[:, c0:c0 + w_], rhs=wt_t[j][:],
                                start=True, stop=False)
                            nc.tensor.matmul(
                                ps[:w_, (j - 1) * F:j * F],
                                lhsT=ones_t[:, :w_],
                                rhs=b16_t[j][:],
                                start=False, stop=True)
                        eng = nc.vector if (t + s) % 2 == 0 else nc.scalar
                        if eng is nc.vector:
                            nc.vector.tensor_copy(st[:w_, s, :], ps[:w_, :])
                        else:
                            nc.scalar.copy(st[:w_, s, :], ps[:w_, :])
                    nc.sync.dma_start(
                        table[r0:r1, :].rearrange("(b a) f -> a b f", a=P),
                        st[:, :nsub, :])
            # ---- SpMM machinery ------------------------------------------
            def spmm(src_w0, src_w1, fdim, dst_bufs, gdt, stg_dts):
                """Gathers stream GM-chunk slices of the global block-major
                chunk stream per window; segment matmuls accumulate per
                block in PSUM; batched scatter-add to pre-zeroed buffers."""
                with tc.tile_pool(name="ga", bufs=4) as gapool, \
                     tc.tile_pool(name="sS", bufs=4) as spool, \
                     tc.tile_pool(name="stg", bufs=2) as stgpool, \
                     tc.tile_pool(name="psC", bufs=4, space="PSUM") as cpsum:
                    wins = [[src_w0, ix0_t, NBLK * K0, [], 0],
                            [src_w1, ix1_t, NBLK * K1, [], 0]]
                    qctr = [0]

                    def ensure_gathers(w, upto_chunk):
                        src_w, ix_t, tot, tiles, _ = wins[w]
                        while wins[w][4] * GM < min(upto_chunk, tot):
                            g = wins[w][4]
                            cg = min(GM, tot - GM * g)
                            ga = gapool.tile([P, GM, fdim], gdt,
                                             tag=f"ga{w}", name=f"ga{w}_{g}")
                            nc.gpsimd.dma_gather(
                                ga[:, :cg, :], src_w,
                                ix_t[:, g * GM * 8: g * GM * 8 + cg * 8],
                                num_idxs=cg * P, num_idxs_reg=cg * P,
                                elem_size=fdim, queue_num=qctr[0] % 4)
                            qctr[0] += 1
                            tiles.append(ga)
                            wins[w][4] += 1

                    stgs = None
                    for b in range(NBLK):
                        g_s, c_s = b // SG, b % SG
                        nb = min(SG, NBLK - g_s * SG)
                        if c_s == 0:
                            stgs = [stgpool.tile([P, SG, F], stg_dts[i],
                                                 tag=f"stg{i}",
                                                 name=f"stg{i}_{g_s}")
                                    for i in range(len(dst_bufs))]
                        ensure_gathers(0, (b + 1) * K0)
                        ensure_gathers(1, (b + 1) * K1)
                        ps = cpsum.tile([P, fdim], f32, space="PSUM")
                        for k in range(K):
                            S = spool.tile([P, P], gdt, tag="S")
                            mo = (b * K + k) * 2
                            nc.vector.tensor_scalar(
                                out=S[:], in0=iota_t[:],
                                scalar1=meta_t[:, mo:mo + 1],
                                scalar2=meta_t[:, mo + 1:mo + 2],
                                op0=mybir.AluOpType.is_equal,
                                op1=mybir.AluOpType.mult)
                            if k < K0:
                                gk = b * K0 + k
                                rhs = wins[0][3][gk // GM][:, gk % GM, :]
                            else:
                                gk = b * K1 + (k - K0)
                                rhs = wins[1][3][gk // GM][:, gk % GM, :]
                            nc.tensor.matmul(ps[:], lhsT=S[:], rhs=rhs,
                                             start=(k == 0),
                                             stop=(k == K - 1))
                        for i, (dst, coff) in enumerate(dst_bufs):
                            nc.vector.tensor_copy(stgs[i][:, c_s, :],
                                                  ps[:, coff:coff + F])
                        if c_s == nb - 1:
                            for i, (dst, coff) in enumerate(dst_bufs):
                                nc.gpsimd.dma_scatter_add(
                                    dst, stgs[i][:, :nb, :],
                                    sid_t[:, g_s * SG * 8:
                                          g_s * SG * 8 + nb * 8],
                                    num_idxs=nb * P, num_idxs_reg=nb * P,
                                    elem_size=F, queue_num=qctr[0] % 4)
                                qctr[0] += 1

            # ---- Phase C: SpMM1 over table -> out1, z2s ------------------
            if "C" in phases:
                spmm(table[:NW0, :], table[cfg.WIN:N, :], 2 * F,
                     [(out1_buf[:], 0), (z2s[:], F)], f16, [f32, f16])

            # ---- Phase D: AllGather z2 shards ----------------------------
            if "D" in phases:
                nc.gpsimd.collective_compute(
                    "AllGather", mybir.AluOpType.bypass,
                    replica_groups=[list(range(NC))],
                    ins=[z2s[0:NS, :]], outs=[z2t[:]],
                )

            # ---- Phase E: SpMM2 over z2 table -> out2 --------------------
            if "E" in phases:
                spmm(z2t[:NW0, :], z2t[cfg.WIN:N, :], F,
                     [(out2_buf[:], 0)], f16, [f32])

    nc.compile()
    return nc


_CACHE = {}


def _get_program(cfg, phases="ABCDE"):
    key = (cfg.N, cfg.F, cfg.NC, cfg.K0, cfg.K1, phases)
    if key not in _CACHE:
        _CACHE[key] = _build_program(cfg, phases)
    return _CACHE[key]


def _prepare(x, edge_weight, W, b, row, col, n_cores=8):
    N, F = np.asarray(x).shape
    row = np.asarray(row).astype(np.int64)
    col = np.asarray(col).astype(np.int64)
    w = np.asarray(edge_weight).astype(np.float32)
    x = np.asarray(x).astype(np.float32)
    W = np.asarray(W).astype(np.float32)
    b = np.asarray(b).astype(np.float32)

    ns = N // n_cores
    core_of = row // ns
    cfg0 = Cfg(N, F, n_cores, 1, 1)
    pcs = []
    for m in range(n_cores):
        sel = np.where(core_of == m)[0]
        pcs.append(_precompute_core(row[sel] - m * ns, col[sel], w[sel], cfg0))
    k0 = max(pc["k0"] for pc in pcs)
    k1 = max(pc["k1"] for pc in pcs)
    cfg = Cfg(N, F, n_cores, k0, k1)

    npad = cfg.NPAD
    xT16 = np.zeros((F, npad), np.float16)
    xT16[:, :N] = x.T.astype(np.float16)
    WT = np.ascontiguousarray(
        np.transpose(W, (0, 2, 1))).reshape(3 * F, F).astype(np.float16)
    BB = np.ascontiguousarray(
        np.broadcast_to(b[:, None, :], (3, P, F))).reshape(3 * P, F)
    B16 = np.ascontiguousarray(b.astype(np.float16))       # [3, F]
    iota = np.tile(np.arange(P, dtype=np.float16), (P, 1))

    in_maps = []
    for m in range(n_cores):
        enc = _encode_core(pcs[m], cfg)
        xs = np.zeros((F, cfg.NBLK * P), np.float16)
        xs[:, :ns] = xT16[:, m * ns:(m + 1) * ns]
        in_maps.append(dict(
            xT16=xT16, xsT=xs, WT=WT, BB=BB, B16=B16, iota=iota,
            idx0=enc["idx0"], idx1=enc["idx1"], meta=enc["meta"],
            sid=enc["sid"],
        ))
    return cfg, in_maps


def kernel(x, edge_weight, W, b, row, col):
    n_cores = 8
    N, F = np.asarray(x).shape
    ns = N // n_cores
    cfg, in_maps = _prepare(x, edge_weight, W, b, row, col, n_cores)
    nc = _get_program(cfg)
    res = bass_utils.run_bass_kernel_spmd(nc, in_maps,
                                          core_ids=list(range(n_cores)))
    outs = []
    for m in range(n_cores):
        r = res.results[m]
        outs.append(np.concatenate(
            [r["y0"][:ns], r["out1"][:ns], r["out2"][:ns]], axis=1))
    return np.concatenate(outs, axis=0).astype(np.float32)



# revision 8
# speedup vs baseline: 2.0057x; 2.0057x over previous
"""MixHop layer (3 hops) on 8 Trainium2 NeuronCores.

out = concat_j [ adj_t^j @ (x @ W_j.T + b_j) ]   for j = 0,1,2

Strategy (destination sharding, one SPMD program on 8 cores):
  - Each core owns N/8 destination rows and the edges pointing into them
    (edges grouped on the host into degree-balanced blocks of 128 dests).
  - Phase A: y0 = x@W0.T+b0 for the own shard (fp32).
  - Phase B: every core (redundantly) projects x -> [y1|y2] table [N,256]
    fp16 with PE matmuls (x passed host-transposed and host-cast to fp16).
  - Phase C (SpMM1): dma_gather 512B fp16 table rows per in-edge (block-major
    global chunk stream, <=1024 ids per gather), build the one-hot*weight
    segment matrix S on device (one tensor_scalar is_equal+mult against an
    iota tile), segment-sum via PE matmuls accumulated in PSUM.  Cols 0:128
    -> out1 (fp32), cols 128:256 -> z2 shard (fp16); both written with
    batched dma_scatter_add into pre-zeroed buffers.
  - Phase D: AllGather z2 shards -> full z2 table [N,128] fp16.
  - Phase E (SpMM2): same edge structure gathers z2 -> out2 (fp32).
All per-core variation (indices, segment data, scatter rows) is carried as
input data so a single program serves all cores.  fp16 is used only for
gather-table payloads and the segment matrix (PSUM accumulation stays fp32);
measured end-to-end error vs the fp32 reference is ~5e-4 max-rel.
"""

import sys

sys.path.insert(0, "/opt/trn_rl_repo")

import heapq
import os

import numpy as np

import concourse.bass as bass
import concourse.tile as tile
from concourse import bacc, mybir
from concourse import bass_utils

P = 128


class Cfg:
    def __init__(self, n_nodes, n_feat, n_cores, k0max, k1max):
        assert n_nodes % n_cores == 0
        self.N = n_nodes
        self.F = n_feat
        self.NC = n_cores
        self.NS = n_nodes // n_cores          # dests per core
        self.NBLK = -(-self.NS // P)          # blocks per core
        self.K0 = k0max                       # window-0 chunks per block
        self.K1 = k1max                       # window-1 chunks per block
        self.K = k0max + k1max
        self.GMAX = 8                         # chunks per dma_gather (<=1024 ids)
        self.SGRP = 8                         # blocks per dma_scatter_add
        self.NSG = -(-self.NBLK // self.SGRP)
        self.NG0 = -(-(self.NBLK * k0max) // self.GMAX)   # win0 gathers/pass
        self.NG1 = -(-(self.NBLK * k1max) // self.GMAX)
        self.WIN = 32768 if n_nodes > 32768 else max(P, n_nodes // 2)
        self.NPAD = -(-n_nodes // 512) * 512  # table rows (512-aligned)


def _balanced_blocks(local_dest, ns, nblk):
    """Assign dests 0..ns-1 to nblk blocks of <=P slots, balancing edge
    counts.  Returns (block_of[ns], pos_of[ns], ids[P, nblk])."""
    deg = np.bincount(local_dest, minlength=ns)
    order = np.argsort(-deg, kind="stable")
    heap = [(0, 0, b) for b in range(nblk)]
    heapq.heapify(heap)
    block_of = np.empty(ns, np.int32)
    pos_of = np.empty(ns, np.int32)
    for d in order:
        while True:
            load, cnt, b = heapq.heappop(heap)
            if cnt < P:
                break
        block_of[d] = b
        pos_of[d] = cnt
        heapq.heappush(heap, (load + int(deg[d]), cnt + 1, b))
    # slot p of block b -> local output row (trash rows ns+p for empty slots)
    ids = np.empty((P, nblk), np.int32)
    for p in range(P):
        ids[p, :] = ns + p
    ids[pos_of, block_of] = np.arange(ns, dtype=np.int32)
    return block_of, pos_of, ids


def _precompute_core(r_loc, c_glob, w, cfg):
    ns, nblk = cfg.NS, cfg.NBLK
    block_of, pos_of, ids = _balanced_blocks(r_loc, ns, nblk)
    b_e = block_of[r_loc]
    dl_e = pos_of[r_loc]
    win_e = (c_glob >= cfg.WIN).astype(np.int64)
    order = np.lexsort((np.arange(len(r_loc)), win_e, b_e))
    b_s, win_s, dl_s, c_s, w_s = (
        b_e[order], win_e[order], dl_e[order], c_glob[order], w[order])
    key = b_s * 2 + win_s
    cnt = np.bincount(key, minlength=nblk * 2).reshape(nblk, 2)
    k0need = max(1, int(np.ceil(cnt[:, 0].max() / P))) if len(r_loc) else 1
    k1need = max(1, int(np.ceil(cnt[:, 1].max() / P))) if len(r_loc) else 1
    return dict(b=b_s, win=win_s, dl=dl_s, c=c_s, w=w_s, cnt=cnt, ids=ids,
                k0=k0need, k1=k1need)


def _encode_core(pc, cfg):
    """Device input arrays for one core, given global K0/K1."""
    nblk, K0, K1, K = cfg.NBLK, cfg.K0, cfg.K1, cfg.K
    cnt = pc["cnt"]
    idx0 = np.zeros((nblk, K0 * P), np.int16)     # padded edge ids (win0)
    idx1 = np.zeros((nblk, K1 * P), np.int16)
    meta = np.zeros((P, nblk, K, 2), np.float32)  # (local dest, weight)
    starts = np.zeros(nblk * 2, np.int64)
    starts[1:] = np.cumsum(cnt.reshape(-1))[:-1]
    key = pc["b"] * 2 + pc["win"]
    iw = np.arange(len(key)) - starts[key]        # index within (b, win)
    b, win, dl, c, w = pc["b"], pc["win"], pc["dl"], pc["c"], pc["w"]
    m0 = win == 0
    idx0[b[m0], iw[m0]] = c[m0].astype(np.int16)
    m1 = ~m0
    idx1[b[m1], iw[m1]] = (c[m1] - cfg.WIN).astype(np.int16)
    kk = np.where(m0, iw // P, K0 + iw // P)
    meta[iw % P, b, kk, 0] = dl
    meta[iw % P, b, kk, 1] = w

    # global chunk-stream gather encodings [128, n_gath*GMAX*8]; dma_gather
    # reads logical id i from [i%16, i//16] of its idx window, replicated to
    # all 8 GPSIMD core groups (partitions 0..127).
    GM = cfg.GMAX

    def enc(idx, Kw, n_gath):
        stream = idx.reshape(nblk * Kw * P)
        out = np.zeros((P, n_gath, GM * 8), np.int16)
        for g in range(n_gath):
            cg = min(GM, nblk * Kw - GM * g)
            flat = stream[g * GM * P: g * GM * P + cg * P]
            e = flat.reshape(-1, 16).T
            out[:, g, :cg * 8] = np.tile(e, (8, 1))
        return out.reshape(P, n_gath * GM * 8)

    # batched scatter ids: group g covers SGRP blocks; logical i = c*128+p
    ids = pc["ids"]
    sid = np.zeros((P, cfg.NSG, cfg.SGRP * 8), np.int16)
    for g in range(cfg.NSG):
        nb = min(cfg.SGRP, nblk - g * cfg.SGRP)
        flat = ids[:, g * cfg.SGRP: g * cfg.SGRP + nb].T.reshape(-1)
        e = flat.reshape(-1, 16).T.astype(np.int16)
        sid[:, g, :nb * 8] = np.tile(e, (8, 1))
    return dict(
        idx0=enc(idx0, K0, cfg.NG0), idx1=enc(idx1, K1, cfg.NG1),
        meta=np.ascontiguousarray(meta.reshape(P, nblk * K * 2)),
        sid=np.ascontiguousarray(sid.reshape(P, cfg.NSG * cfg.SGRP * 8)),
    )


def _build_program(cfg, phases="ABCDE"):
    N, F, NC = cfg.N, cfg.F, cfg.NC
    NS, NBLK, K0, K1, K = cfg.NS, cfg.NBLK, cfg.K0, cfg.K1, cfg.K
    NW0 = min(N, cfg.WIN)
    NSP = NS + P                             # out buf rows incl trash
    NTILE_Y0 = NBLK
    NTILE_TAB = -(-N // P)
    f32 = mybir.dt.float32
    f16 = mybir.dt.float16
    GM, NG0, NG1 = cfg.GMAX, cfg.NG0, cfg.NG1
    SG, NSG = cfg.SGRP, cfg.NSG

    nc = bacc.Bacc("TRN2", target_bir_lowering=False, debug=False,
                   enable_asserts=False, num_devices=NC, num_swdge_queues=4,
                   dynamic_dma_scratch_size=65536)

    # ---- inputs ----------------------------------------------------------
    NP_ = cfg.NPAD
    xT16 = nc.dram_tensor("xT16", [F, NP_], f16, kind="ExternalInput").ap()
    xsT = nc.dram_tensor("xsT", [F, NBLK * P], f16, kind="ExternalInput").ap()
    WT = nc.dram_tensor("WT", [3 * F, F], f16, kind="ExternalInput").ap()
    BB = nc.dram_tensor("BB", [3 * P, F], f32, kind="ExternalInput").ap()
    B16 = nc.dram_tensor("B16", [3, F], f16, kind="ExternalInput").ap()
    iota_in = nc.dram_tensor("iota", [P, P], f16, kind="ExternalInput").ap()
    idx0_in = nc.dram_tensor("idx0", [P, NG0 * GM * 8], mybir.dt.int16,
                             kind="ExternalInput").ap()
    idx1_in = nc.dram_tensor("idx1", [P, NG1 * GM * 8], mybir.dt.int16,
                             kind="ExternalInput").ap()
    meta_in = nc.dram_tensor("meta", [P, NBLK * K * 2], f32,
                             kind="ExternalInput").ap()
    sid_in = nc.dram_tensor("sid", [P, NSG * SG * 8], mybir.dt.int16,
                            kind="ExternalInput").ap()

    # ---- outputs / scratch ----------------------------------------------
    y0_buf = nc.dram_tensor("y0", [NBLK * P, F], f32, kind="ExternalOutput").ap()
    out1_buf = nc.dram_tensor("out1", [NSP, F], f32, kind="ExternalOutput").ap()
    out2_buf = nc.dram_tensor("out2", [NSP, F], f32, kind="ExternalOutput").ap()
    table = nc.dram_tensor("table", [NP_, 2 * F], f16, kind="Internal").ap()
    z2s = nc.dram_tensor("z2s", [NSP, F], f16, kind="Internal").ap()
    z2t = nc.dram_tensor("z2t", [N, F], f16, kind="Internal",
                         addr_space="Shared").ap()

    with tile.TileContext(nc) as tc:
        with tc.tile_pool(name="const", bufs=1) as cpool:
            iota_t = cpool.tile([P, P], f16)
            nc.sync.dma_start(iota_t[:], iota_in[:])
            meta_t = cpool.tile([P, NBLK * K * 2], f32)
            nc.sync.dma_start(meta_t[:], meta_in[:])
            ix0_t = cpool.tile([P, NG0 * GM * 8], mybir.dt.int16)
            nc.sync.dma_start(ix0_t[:], idx0_in[:])
            ix1_t = cpool.tile([P, NG1 * GM * 8], mybir.dt.int16)
            nc.sync.dma_start(ix1_t[:], idx1_in[:])
            sid_t = cpool.tile([P, NSG * SG * 8], mybir.dt.int16)
            nc.sync.dma_start(sid_t[:], sid_in[:])
            wt_t = []
            bb_t = []
            for j in range(3):
                wtj = cpool.tile([F, F], f16, tag=f"wt{j}", name=f"wt{j}")
                bbj = cpool.tile([P, F], f32, tag=f"bb{j}", name=f"bb{j}")
                nc.sync.dma_start(wtj[:], WT[j * F:(j + 1) * F, :])
                nc.sync.dma_start(bbj[:], BB[j * P:(j + 1) * P, :])
                wt_t.append(wtj)
                bb_t.append(bbj)
            b16_t = []
            for j in range(3):
                b16j = cpool.tile([1, F], f16, tag=f"b16{j}", name=f"b16{j}")
                nc.sync.dma_start(b16j[:], B16[j:j + 1, :])
                b16_t.append(b16j)
            ones_t = cpool.tile([1, P], f16)
            nc.vector.memset(ones_t[:], 1.0)

            # ---- zero z2s (scatter-add base) -----------------------------
            if "C" in phases:
                with tc.tile_pool(name="zz", bufs=1) as zpool:
                    zt = zpool.tile([P, 2048], f16)
                    nc.vector.memset(zt[:], 0.0)
                    nrow = 0
                    while nrow + 2048 <= NSP:
                        nc.sync.dma_start(
                            z2s[nrow:nrow + 2048, :].rearrange(
                                "(a b) f -> a (b f)", a=P), zt[:])
                        nrow += 2048
                    while nrow + P <= NSP:
                        nc.sync.dma_start(
                            z2s[nrow:nrow + P, :].rearrange(
                                "(a b) f -> a (b f)", a=P), zt[:, :F])
                        nrow += P
                    assert nrow >= NS, (nrow, NS)

            # ---- Phase A: y0 = xs@W0.T + b0 (own shard, fp32 out) --------
            if "A" in phases:
             with tc.tile_pool(name="projA", bufs=3) as apool, \
                  tc.tile_pool(name="psumA", bufs=3, space="PSUM") as apsum:
                for t in range(NTILE_Y0):
                    r0 = t * P
                    r1 = min(NS, r0 + P)
                    w_ = r1 - r0
                    if w_ <= 0:
                        break
                    xt = apool.tile([F, P], f16, tag="xt")
                    nc.sync.dma_start(xt[:, :w_], xsT[:, r0:r1])
                    ps0 = apsum.tile([P, F], f32, space="PSUM")
                    nc.tensor.matmul(ps0[:w_, :], lhsT=xt[:, :w_],
                                     rhs=wt_t[0][:], start=True, stop=True)
                    st0 = apool.tile([P, F], f32, tag="st0")
                    nc.vector.tensor_tensor(out=st0[:w_, :], in0=ps0[:w_, :],
                                            in1=bb_t[0][:w_, :],
                                            op=mybir.AluOpType.add)
                    nc.sync.dma_start(y0_buf[r0:r1, :], st0[:w_, :])

            # ---- Phase B: full fp16 table (replicated per core) ----------
            # 512-node groups: one wide load, 4x2 matmuls (+rank-1 bias
            # matmul), one wide store.  psum->staging copies alternate
            # DVE/ACT to spread engine load.
            if "B" in phases:
             NGRP_B = NP_ // 512
             with tc.tile_pool(name="projB", bufs=3) as bpool, \
                  tc.tile_pool(name="psumB", bufs=4, space="PSUM") as bpsum:
                for t in range(NGRP_B):
                    r0 = t * 512
                    r1 = r0 + 512
                    gw = 512
                    xt = bpool.tile([F, 512], f16, tag="xtb")
                    nc.sync.dma_start(xt[:, :gw], xT16[:, r0:r1])
                    st = bpool.tile([P, 4, 2 * F], f16, tag="stb")
                    nsub = -(-gw // P)
                    for s in range(nsub):
                        c0 = s * P
                        w_ = min(P, gw - c0)
                        ps = bpsum.tile([P, 2 * F], f32, space="PSUM")
                        for j in (1, 2):
                            nc.tensor.matmul(
                                ps[:w_, (j - 1) * F:j * F],
                                lhsT=xt[:, c0:c0 + w_], rhs=wt_t[j][:],
                                start=True, stop=False)
                            nc.tensor.matmul(
                                ps[:w_, (j - 1) * F:j * F],
                                lhsT=ones_t[:, :w_],
                                rhs=b16_t[j][:],
                                start=False, stop=True)
                        eng = nc.vector if (t + s) % 2 == 0 else nc.scalar
                        if eng is nc.vector:
                            nc.vector.tensor_copy(st[:w_, s, :], ps[:w_, :])
                        else:
                            nc.scalar.copy(st[:w_, s, :], ps[:w_, :])
                    nc.sync.dma_start(
                        table[r0:r1, :].rearrange("(b a) f -> a b f", a=P),
                        st[:, :nsub, :])
            # ---- SpMM machinery ------------------------------------------
            def spmm(src_w0, src_w1, fdim, dst_bufs, gdt, stg_dts):
                """Gathers stream GM-chunk slices of the global block-major
                chunk stream per window; segment matmuls accumulate per
                block in PSUM; batched scatter-add to pre-zeroed buffers."""
                with tc.tile_pool(name="ga", bufs=6) as gapool, \
                     tc.tile_pool(name="sS", bufs=4) as spool, \
                     tc.tile_pool(name="stg", bufs=2) as stgpool, \
                     tc.tile_pool(name="psC", bufs=4, space="PSUM") as cpsum:
                    wins = [[src_w0, ix0_t, NBLK * K0, [], 0],
                            [src_w1, ix1_t, NBLK * K1, [], 0]]
                    qctr = [0]

                    def ensure_gathers(w, upto_chunk):
                        src_w, ix_t, tot, tiles, _ = wins[w]
                        while wins[w][4] * GM < min(upto_chunk, tot):
                            g = wins[w][4]
                            cg = min(GM, tot - GM * g)
                            ga = gapool.tile([P, GM, fdim], gdt,
                                             tag=f"ga{w}", name=f"ga{w}_{g}")
                            nc.gpsimd.dma_gather(
                                ga[:, :cg, :], src_w,
                                ix_t[:, g * GM * 8: g * GM * 8 + cg * 8],
                                num_idxs=cg * P, num_idxs_reg=cg * P,
                                elem_size=fdim, queue_num=qctr[0] % 4)
                            qctr[0] += 1
                            tiles.append(ga)
                            wins[w][4] += 1

                    stgs = None
                    for b in range(NBLK):
                        g_s, c_s = b // SG, b % SG
                        nb = min(SG, NBLK - g_s * SG)
                        if c_s == 0:
                            stgs = [stgpool.tile([P, SG, F], stg_dts[i],
                                                 tag=f"stg{i}",
                                                 name=f"stg{i}_{g_s}")
                                    for i in range(len(dst_bufs))]
                        ensure_gathers(0, (b + 1) * K0)
                        ensure_gathers(1, (b + 1) * K1)
                        ps = cpsum.tile([P, fdim], f32, space="PSUM")
                        for k in range(K):
                            S = spool.tile([P, P], gdt, tag="S")
                            mo = (b * K + k) * 2
                            nc.vector.tensor_scalar(
                                out=S[:], in0=iota_t[:],
                                scalar1=meta_t[:, mo:mo + 1],
                                scalar2=meta_t[:, mo + 1:mo + 2],
                                op0=mybir.AluOpType.is_equal,
                                op1=mybir.AluOpType.mult)
                            if k < K0:
                                gk = b * K0 + k
                                rhs = wins[0][3][gk // GM][:, gk % GM, :]
                            else:
                                gk = b * K1 + (k - K0)
                                rhs = wins[1][3][gk // GM][:, gk % GM, :]
                            nc.tensor.matmul(ps[:], lhsT=S[:], rhs=rhs,
                                             start=(k == 0),
                                             stop=(k == K - 1))
                        for i, (dst, coff) in enumerate(dst_bufs):
                            nc.vector.tensor_copy(stgs[i][:, c_s, :],
                                                  ps[:, coff:coff + F])
                        if c_s == nb - 1:
                            for i, (dst, coff) in enumerate(dst_bufs):
                                nc.gpsimd.dma_scatter_add(
                                    dst, stgs[i][:, :nb, :],
                                    sid_t[:, g_s * SG * 8:
                                          g_s * SG * 8 + nb * 8],
                                    num_idxs=nb * P, num_idxs_reg=nb * P,
                                    elem_size=F, queue_num=qctr[0] % 4)
                                qctr[0] += 1

            # ---- Phase C: SpMM1 over table -> out1, z2s ------------------
            if "C" in phases:
                spmm(table[:NW0, :], table[cfg.WIN:N, :], 2 * F,
                     [(out1_buf[:], 0), (z2s[:], F)], f16, [f32, f16])

            # ---- Phase D: AllGather z2 shards ----------------------------
            if "D" in phases:
                nc.gpsimd.collective_compute(
                    "AllGather", mybir.AluOpType.bypass,
                    replica_groups=[list(range(NC))],
                    ins=[z2s[0:NS, :]], outs=[z2t[:]],
                )

            # ---- Phase E: SpMM2 over z2 table -> out2 --------------------
            if "E" in phases:
                spmm(z2t[:NW0, :], z2t[cfg.WIN:N, :], F,
                     [(out2_buf[:], 0)], f16, [f32])

    nc.compile()
    return nc


_CACHE = {}


def _get_program(cfg, phases="ABCDE"):
    key = (cfg.N, cfg.F, cfg.NC, cfg.K0, cfg.K1, phases)
    if key not in _CACHE:
        _CACHE[key] = _build_program(cfg, phases)
    return _CACHE[key]


def _prepare(x, edge_weight, W, b, row, col, n_cores=8):
    N, F = np.asarray(x).shape
    row = np.asarray(row).astype(np.int64)
    col = np.asarray(col).astype(np.int64)
    w = np.asarray(edge_weight).astype(np.float32)
    x = np.asarray(x).astype(np.float32)
    W = np.asarray(W).astype(np.float32)
    b = np.asarray(b).astype(np.float32)

    ns = N // n_cores
    core_of = row // ns
    cfg0 = Cfg(N, F, n_cores, 1, 1)
    pcs = []
    for m in range(n_cores):
        sel = np.where(core_of == m)[0]
        pcs.append(_precompute_core(row[sel] - m * ns, col[sel], w[sel], cfg0))
    k0 = max(pc["k0"] for pc in pcs)
    k1 = max(pc["k1"] for pc in pcs)
    cfg = Cfg(N, F, n_cores, k0, k1)

    npad = cfg.NPAD
    xT16 = np.zeros((F, npad), np.float16)
    xT16[:, :N] = x.T.astype(np.float16)
    WT = np.ascontiguousarray(
        np.transpose(W, (0, 2, 1))).reshape(3 * F, F).astype(np.float16)
    BB = np.ascontiguousarray(
        np.broadcast_to(b[:, None, :], (3, P, F))).reshape(3 * P, F)
    B16 = np.ascontiguousarray(b.astype(np.float16))       # [3, F]
    iota = np.tile(np.arange(P, dtype=np.float16), (P, 1))

    in_maps = []
    for m in range(n_cores):
        enc = _encode_core(pcs[m], cfg)
        xs = np.zeros((F, cfg.NBLK * P), np.float16)
        xs[:, :ns] = xT16[:, m * ns:(m + 1) * ns]
        in_maps.append(dict(
            xT16=xT16, xsT=xs, WT=WT, BB=BB, B16=B16, iota=iota,
            idx0=enc["idx0"], idx1=enc["idx1"], meta=enc["meta"],
            sid=enc["sid"],
        ))
    return cfg, in_maps


def kernel(x, edge_weight, W, b, row, col):
    n_cores = 8
    N, F = np.asarray(x).shape
    ns = N // n_cores
    cfg, in_maps = _prepare(x, edge_weight, W, b, row, col, n_cores)
    nc = _get_program(cfg)
    res = bass_utils.run_bass_kernel_spmd(nc, in_maps,
                                          core_ids=list(range(n_cores)))
    outs = []
    for m in range(n_cores):
        r = res.results[m]
        outs.append(np.concatenate(
            [r["y0"][:ns], r["out1"][:ns], r["out2"][:ns]], axis=1))
    return np.concatenate(outs, axis=0).astype(np.float32)

